# revision 40
# baseline (speedup 1.0000x reference)
"""Causal multi-head attention on 8 Trainium2 NeuronCores.

Sharding: tensor-parallel across heads. 16 heads, 8 cores -> 2 heads/core.
Each core gets the full (pre-transposed, bf16) activations qT/kT/vT and its
slice of the projection weights; it computes the partial output
concat_c @ WoT_c [B*T, C]; the host sums the 8 partials (the "all-reduce
after the output projection").

Numerics: bf16 operands on the whole matmul path (per-element err ~0.4%
passes the 2e-2 gate with ~5x margin; fp8 anywhere fails it), fp32 PSUM
accumulation, softmax in fp32.  Measured end-to-end rel err ~4e-3.

Device math per core:
  KHT/QHT = W @ xT    [128 head-dims, 2048] bf16, 8 K-chunk accumulation
  VH      = xT.T @ Wv [128 tokens, 128 head-dims] per key block -- computed
            directly in key-major orientation (no PE transpose), with a
            ones column appended so the PV matmul also emits softmax
            denominators.
  Per (batch b, 512-wide query group qg), kb = key block, both heads l:
      ST[l]  = KH_l @ QHT_l block  [128 keys, 512-c0 queries] into paired
               PSUM banks (c0 = 128*d causal left-trim on diagonal blocks)
      P      = exp(ST/8)           ONE ACT per kb covering both heads
      causal: gpsimd.affine_select zeroes the 128-wide triangle block only
      OT[l] += [VH_l|1].T @ P[l]   [65, 512-c0] PSUM accumulate
    (PV lags the ST/exp chain by up to 6 kb)
    OT[0:64] *= 1/OT[64]  (DVE reciprocal + gpsimd partition_broadcast)
  OUT rows = OTall_b.T @ WoT_c, bf16 out.

Schedule (v2, 145us -> ~12xus): the attention chains are ACT-bound in the
cost model -- exp streams at the same rate as ST+PV on PE (1.67 ns/query
column both), so the ~185ns per-exp fixed overhead surfaces as a PE stall
every key block.  v2 therefore feeds the chains PE "filler": every
projection / output-projection matmul is emitted as a closure and
interleaved between chain steps, so PE always has ~15-25% more work per kb
than ACT and the exp overhead hides completely:
  - v-projections of group g fill batch b's own chain of group g (their
    key blocks are only consumed in the PV drain, after the fillers ran);
  - kq-projections of group n+1 fill chain(b1, n) (their input DMAs were
    issued at iteration-n start, a full chain earlier);
  - output projections of group n-1 fill chain(b0, n); outproj(b0, 3)
    fills chain(b1, 3) so only outproj(b1, 3) remains as the tail.
  Out-DMAs are issued from the DVE queue right after their evacuation
  copy (no extra sem wait, SP stays free for input prefetch).  Weights
  stream with a chunk-0 sliver of Wk first so the first projection matmul
  starts ~1us earlier.
"""

from collections import deque

import numpy as np

B, T, C = 2, 2048, 1024
H, DK = 16, 64
NCORES = 8
HL = H // NCORES          # local heads per core = 2
LD = HL * DK              # local head dims per core = 128
N = B * T                 # 4096 rows
KCH = C // 128            # 8 contraction chunks
QG = T // 512             # 4 query groups per batch
KB = T // 128             # 16 key blocks per batch

LAST_RESULTS = None       # BassKernelResults of the most recent run (for test.py)
MM_LOG = []               # phase tag per emitted PE matmul (profiling aid)


def _build_program():
    import concourse.tile as tile
    import concourse.mybir as mybir
    from concourse import bacc
    from contextlib import ExitStack

    f32 = mybir.dt.float32
    bf16 = mybir.dt.bfloat16
    EXP = mybir.ActivationFunctionType.Exp

    nc = bacc.Bacc("TRN2", target_bir_lowering=False, debug=False, num_devices=NCORES)
    # activations pre-chunked on host: x[p, kk, t] = xT[kk*128+p, t]
    qT_d = nc.declare_dram_parameter("qT", [128, KCH, N], bf16, isOutput=False)
    kT_d = nc.declare_dram_parameter("kT", [128, KCH, N], bf16, isOutput=False)
    vT_d = nc.declare_dram_parameter("vT", [128, KCH, N], bf16, isOutput=False)
    # weights pre-chunked on host: w[p, kk, l] = WT[kk*128+p, l]
    wq_d = nc.declare_dram_parameter("wqT", [128, KCH, LD], bf16, isOutput=False)
    wk_d = nc.declare_dram_parameter("wkT", [128, KCH, LD], bf16, isOutput=False)
    wv_d = nc.declare_dram_parameter("wvT", [128, KCH, LD], bf16, isOutput=False)
    wo_d = nc.declare_dram_parameter("woT", [LD, C], bf16, isOutput=False)
    # out[p, blk, c] = row blk*128+p of the [N, C] output (host transposes);
    # this layout lets one DMA cover a whole 512-row group from the SBUF
    # staging tile with matching iteration order
    out_d = nc.declare_dram_parameter("out", [128, N // 128, C], bf16,
                                      isOutput=True)

    MM_LOG.clear()
    _cur = [""]

    def MM(*args, **kw):
        MM_LOG.append(_cur[0])
        return nc.tensor.matmul(*args, **kw)

    with ExitStack() as ctx:
        tc = ctx.enter_context(tile.TileContext(nc))
        const = ctx.enter_context(tc.tile_pool(name="const", bufs=1))
        persist = ctx.enter_context(tc.tile_pool(name="persist", bufs=1))
        xpool = ctx.enter_context(tc.tile_pool(name="xt", bufs=5))
        ppool = ctx.enter_context(tc.tile_pool(name="p", bufs=11))
        stg = ctx.enter_context(tc.tile_pool(name="stg", bufs=1))
        spool = ctx.enter_context(tc.tile_pool(name="small", bufs=1))
        stps = ctx.enter_context(tc.tile_pool(name="stps", bufs=2, space="PSUM"))
        otps = ctx.enter_context(tc.tile_pool(name="otps", bufs=2, space="PSUM"))
        mps = ctx.enter_context(tc.tile_pool(name="mps", bufs=2, space="PSUM"))

        # ---- weights: each DMA is emitted just before its first consumer's
        # load call; wk additionally ships its chunk 0 as a separate sliver
        # so the very first projection matmul has its stationary operand
        # ~1.3us earlier ----
        wq = const.tile([128, KCH, LD], bf16)
        wk = const.tile([128, KCH, LD], bf16)
        wv = const.tile([128, KCH, LD], bf16)
        wo = const.tile([128, C], bf16)
        w_dram_of = {id(wk): wk_d, id(wv): wv_d, id(wq): wq_d, id(wo): wo_d}
        w_loaded = set()

        def load_weight(w_t):
            if id(w_t) not in w_loaded:
                w_loaded.add(id(w_t))
                if w_t is wk:
                    nc.sync.dma_start(w_t[:, 0:1, :], w_dram_of[id(w_t)][:, 0:1, :])
                    nc.sync.dma_start(w_t[:, 1:KCH, :], w_dram_of[id(w_t)][:, 1:KCH, :])
                else:
                    nc.sync.dma_start(w_t[:], w_dram_of[id(w_t)][:])

        # per-batch persistent activations
        qht = [persist.tile([128, T], bf16, name=f"qht{b}") for b in range(B)]
        kht = [persist.tile([128, T], bf16, name=f"kht{b}") for b in range(B)]
        otall = [persist.tile([128, T], bf16, name=f"otall{b}") for b in range(B)]
        # VH blocks [keys, dk] per (key block, local head), ones col appended
        vh = [persist.tile([128, KB, HL, 65], bf16, name=f"vh{b}") for b in range(B)]
        for b in range(B):
            nc.vector.memset(vh[b][:, :, :, 64:65], 1.0)
        # ones column for the K=1 broadcast matmul in the softmax rescale
        ones1 = const.tile([1, 64], bf16)
        nc.vector.memset(ones1[:], 1.0)

        loads = {}

        def _load(b, n, keys, eng=None):
            # DMA issue only (SP queue by default), two half-DMAs per tensor
            cols = slice(b * T + n * 512, b * T + (n + 1) * 512)
            tiles = loads.setdefault((b, n), {})
            for key, w_t, src in keys:
                load_weight(w_t)
                xh = [xpool.tile([128, KCH // 2, 512], bf16,
                                 name=f"xh_{key}{i}", tag=f"xh_{key}{i}",
                                 bufs=4 if key == "v" else None)
                      for i in range(2)]
                for i in range(2):
                    (eng or nc.sync).dma_start(
                        xh[i][:], src[:, i * (KCH // 2):(i + 1) * (KCH // 2), cols])
                tiles[key] = xh

        def load_kq(b, n, eng=None):
            if "k" in loads.get((b, n), {}):
                _load(b, n, (("q", wq, qT_d),), eng)
            else:
                _load(b, n, (("k", wk, kT_d), ("q", wq, qT_d)), eng)

        def load_v(b, n, eng=None):
            _load(b, n, (("v", wv, vT_d),), eng)

        # ---------- filler generators: each closure emits ~one matmul ----------
        def kq_fillers(b, n):
            # k/q projection of one 512-token group: 16 closures of one
            # 512-col chunk matmul each; the 8th chunk attaches the DVE evac
            fs = []
            for key, w_t, dst in (("k", wk, kht[b]), ("q", wq, qht[b])):
                holder = {}

                def go(kk, key=key, w_t=w_t, dst=dst, holder=holder):
                    tiles = loads[(b, n)]
                    if kk == 0:
                        holder["ps"] = mps.tile([128, 512], f32, tag="mm",
                                                name="kqps")
                    psb = holder["ps"]
                    xh = tiles[key]
                    src = xh[kk // (KCH // 2)][:, kk % (KCH // 2), :]
                    _cur[0] = f"kq{key}{b}g{n}"
                    MM(psb[:], w_t[:, kk, :], src,
                       start=(kk == 0), stop=(kk == KCH - 1))
                    if kk == KCH - 1:
                        nc.vector.tensor_copy(dst[:, n * 512:(n + 1) * 512], psb[:])

                fs += [(lambda kk=kk, go=go: go(kk)) for kk in range(KCH)]
            return fs

        def v_fillers(b, n):
            # v projection in key-major orientation: per 128-token key block,
            # two closures of 4 accumulation chunks each (~213ns PE apiece)
            fs = []
            for j in range(4):
                holder = {}

                def go(half, j=j, holder=holder):
                    vxh = loads[(b, n)]["v"]
                    if half == 0:
                        holder["ps"] = mps.tile([128, 128], f32, tag="mm",
                                                name="vps")
                    psb = holder["ps"]
                    _cur[0] = f"v{b}g{n}"
                    for kk in range(half * 4, half * 4 + 4):
                        MM(
                            psb[:],
                            vxh[kk // (KCH // 2)][:, kk % (KCH // 2),
                                                  j * 128:(j + 1) * 128],
                            wv[:, kk, :],
                            start=(kk == 0), stop=(kk == KCH - 1))
                    if half == 1:
                        kb = 4 * n + j
                        nc.vector.tensor_copy(vh[b][:, kb, :, 0:64], psb[:])

                fs += [(lambda half=half, go=go: go(half)) for half in range(2)]
            return fs

        def outproj_fillers(b, qg, tail=False):
            # output projection for one 512-row group: 8 closures of one
            # [128,512] matmul each, evacuated into an SBUF staging tile
            # (evacs round-robin DVE/DVE/Pool so no single engine paces
            # them); the caller emits finish() after the consuming chain --
            # ONE big out-DMA per (b, group), so out-DMAs are never
            # latency-critical behind the input-prefetch DMA queue.
            # In the drain tail, copies alternate DVE/ACT (ACT is idle) and
            # each row-block is DMA'd as soon as its copies land.
            load_weight(wo)
            q0 = qg * 512
            blk0 = b * (T // 128) + qg * 4
            holder = {}
            fs = []
            for rt in range(4):
                for nn in range(2):
                    def go(rt=rt, nn=nn):
                        row0 = q0 + rt * 128
                        if rt == 0 and nn == 0:
                            # stage0 double-buffered: outproj(b0, 3) runs as
                            # filler inside chain(b1, 3), right after
                            # outproj(b0, 2)'s DMA was issued
                            holder["st"] = stg.tile([128, 4, 2, 512], bf16,
                                                    tag=f"stage{b}", name="stage",
                                                    bufs=2 if b == 0 else 1)
                        sg = holder["st"]
                        ops = mps.tile([128, 512], f32, tag="mm", name="ops")
                        _cur[0] = f"op{b}g{qg}"
                        MM(ops[:], otall[b][:, row0:row0 + 128],
                           wo[:, nn * 512:(nn + 1) * 512],
                           start=True, stop=True)
                        idx = rt * 2 + nn
                        if tail:
                            # ACT first: it is idle in the drain, and DVE is
                            # still finishing the rescale muls; each half is
                            # DMA'd right after its own copy so the final
                            # transfer is small and starts early
                            if idx % 2 == 0:
                                nc.scalar.copy(sg[:, rt, nn, :], ops[:])
                                nc.scalar.dma_start(
                                    out_d[:, blk0 + rt, nn * 512:(nn + 1) * 512],
                                    sg[:, rt, nn, :])
                            else:
                                nc.vector.tensor_copy(sg[:, rt, nn, :], ops[:])
                                nc.sync.dma_start(
                                    out_d[:, blk0 + rt, nn * 512:(nn + 1) * 512],
                                    sg[:, rt, nn, :])
                        else:
                            nc.vector.tensor_copy(sg[:, rt, nn, :], ops[:])

                    fs.append(go)

            def finish():
                nc.sync.dma_start(out_d[:, blk0:blk0 + 4, :], holder["st"][:])

            return fs, (None if tail else finish)

        def merge_fillers(*streams):
            # proportional merge: each stream is spread uniformly over the
            # result, so same-kind fillers (which share an mps allocation
            # cadence) never arrive as a burst that outruns their evacs
            items = []
            for si, s in enumerate(streams):
                s = list(s)
                for k, f in enumerate(s):
                    items.append(((k + 0.5) / len(s), si, k, f))
            items.sort(key=lambda t: (t[0], t[1]))
            return [f for _, _, _, f in items]

        def attention_qg(b, qg, fillers=()):
            fillers = deque(fillers)
            nf = len(fillers)
            q0 = qg * 512
            nkb = 4 * qg + 4
            # lag 10: PV(kb=0) starts late enough that the previous chain's
            # rescale (which reads the otp tiles this chain's first PV
            # recycles, otps bufs=2) has long finished -- the recip ->
            # partition_broadcast -> mul chain is ~4us when DVE is loaded
            lag = min(10, nkb)
            otp = [otps.tile([65, 512], f32, tag="otp", name=f"otp_{b}_{qg}_{l}")
                   for l in range(HL)]
            ps = []

            def pv(kb):
                p, c0 = ps[kb]
                _cur[0] = f"pv{b}q{qg}k{kb}"
                for l in range(HL):
                    MM(otp[l][:, c0:512], vh[b][:, kb, l, :],
                       p[:, l, c0:512],
                       start=(kb == 0), stop=(kb == nkb - 1))

            for kb in range(nkb):
                d = kb - 4 * qg
                c0 = 128 * d if d > 0 else 0
                st = stps.tile([128, HL, 512], f32, tag="st",
                               name=f"st_{b}_{qg}_{kb}")
                _cur[0] = f"st{b}q{qg}k{kb}"
                for l in range(HL):
                    hs = slice(l * 64, (l + 1) * 64)
                    MM(
                        st[:, l, c0:512],
                        kht[b][hs, kb * 128:(kb + 1) * 128],
                        qht[b][hs, q0 + c0: q0 + 512],
                        start=True, stop=True)
                p = ppool.tile([128, HL, 512], bf16, tag="p",
                               name=f"p_{b}_{qg}_{kb}")
                nc.scalar.activation(p[:, :, c0:512], st[:, :, c0:512],
                                     EXP, scale=0.125)
                if d >= 0:
                    # zero keys below the causal diagonal; only the 128-wide
                    # triangle block [c0:c0+128] can violate causality
                    nc.gpsimd.affine_select(
                        out=p[:, :, c0:c0 + 128], in_=p[:, :, c0:c0 + 128],
                        compare_op=mybir.AluOpType.is_ge,
                        fill=0.0, base=0, channel_multiplier=-1,
                        pattern=[[0, HL], [1, 128]])
                ps.append((p, c0))
                if kb >= lag:
                    pv(kb - lag)
                # evenly-spread PE filler for this step
                take = (nf * (kb + 1)) // nkb - (nf * kb) // nkb
                for _ in range(take):
                    fillers.popleft()()
            for t in range(lag, 0, -1):
                pv(nkb - t)
            while fillers:
                fillers.popleft()()

            # two-part rescale: part A computes the denominators' reciprocals
            # (DVE); part B broadcasts them across 64 partitions with a K=1
            # matmul against the ones column (213ns on PE -- keeps Pool free
            # for affine_selects) and applies the muls (DVE).  Consumed as
            # two separate filler slots a few steps apart so the PE matmul
            # never waits on the recips.
            recips = []

            def part_a():
                for l in range(HL):
                    recip = spool.tile([1, 512], f32, tag=f"recip{l}")
                    nc.vector.reciprocal(recip[:], otp[l][64:65, :])
                    recips.append(recip)

            def part_b():
                _cur[0] = f"rs{b}q{qg}"
                for l in range(HL):
                    rep = mps.tile([64, 512], f32, tag="mm", name="rep")
                    MM(rep[:], ones1[:], recips[l][:], start=True, stop=True)
                    with nc.allow_low_precision(reason="bf16 out of f32 softmax"):
                        nc.vector.tensor_mul(
                            otall[b][l * 64:(l + 1) * 64, q0: q0 + 512],
                            otp[l][0:64, :], rep[:])
            return part_a, part_b

        # ---------------- schedule ----------------
        # cold start: k/q of (b0, group 0) stream as fine-grained pieces
        # interleaved with chunk-0 weight slivers, ordered so each
        # projection matmul's operands arrive just before the PE needs them
        # (the DMA path is FIFO: HWDGE + the shared DMA-engine device)
        cols0 = slice(0, 512)
        k00 = [xpool.tile([128, KCH // 2, 512], bf16,
                          name=f"xh_k{i}", tag=f"xh_k{i}") for i in range(2)]
        q00 = [xpool.tile([128, KCH // 2, 512], bf16,
                          name=f"xh_q{i}", tag=f"xh_q{i}") for i in range(2)]
        w_loaded.update((id(wk), id(wq)))
        nc.sync.dma_start(wk[:, 0:1, :], wk_d[:, 0:1, :])
        nc.sync.dma_start(k00[0][:, 0:2, :], kT_d[:, 0:2, cols0])
        nc.sync.dma_start(wk[:, 1:KCH, :], wk_d[:, 1:KCH, :])
        nc.sync.dma_start(k00[0][:, 2:4, :], kT_d[:, 2:4, cols0])
        nc.sync.dma_start(wq[:, 0:1, :], wq_d[:, 0:1, :])
        nc.sync.dma_start(k00[1][:, 0:2, :], kT_d[:, 4:6, cols0])
        nc.sync.dma_start(k00[1][:, 2:4, :], kT_d[:, 6:8, cols0])
        nc.sync.dma_start(q00[0][:], qT_d[:, 0:KCH // 2, cols0])
        nc.sync.dma_start(wq[:, 1:KCH, :], wq_d[:, 1:KCH, :])
        nc.sync.dma_start(q00[1][:], qT_d[:, KCH // 2:KCH, cols0])
        loads[(0, 0)] = {"k": k00, "q": q00}
        load_v(0, 0)                     # (+wv)
        load_kq(1, 0)
        load_v(1, 0)
        load_kq(0, 1)
        load_kq(1, 1)
        load_v(0, 1)
        load_v(1, 1)
        load_weight(wo)

        # iteration 0: kq projections of group 0 run dense (chains need
        # them); v projections are self-fillers inside their own chain;
        # kq(.,1) runs dense after (its data is still streaming in)
        def with_rescale(fillers, resc, frac=0.25):
            # a deferred rescale emits its DVE/Pool ops ~frac into the next
            # chain, where the evacuation queues have slack (at a chain
            # boundary they collide with the new chain's first evacs and
            # stall PE on the mps WAR); lag-10 PV starts keep the otp WAR
            # of the chain after next well clear
            if resc is None:
                return fillers
            fillers = list(fillers)
            fillers.insert(max(1, int(len(fillers) * frac)), resc)
            return fillers

        # NOTE a deferred rescale of chain X MUST be emitted before chain
        # X+1's PV instructions (the otp PSUM slots rotate every chain, and
        # the WAR is only tracked against readers already emitted) -- i.e.
        # it can only ride in the next chain's filler stream, never later.
        for f in kq_fillers(0, 0):
            f()
        r0 = attention_qg(0, 0, v_fillers(0, 0))
        for f in kq_fillers(1, 0):
            f()
        r1 = attention_qg(1, 0, with_rescale(v_fillers(1, 0), r0))
        for f in with_rescale(kq_fillers(0, 1) + kq_fillers(1, 1), r1, 0.3):
            f()
        # group-2 k/q is issued here already (consumed by fillers inside
        # chain(b1,1)); later iterations prefetch group n+2 k/q at their
        # start and group n+1 v after chain b0, keeping the (serialized,
        # FIFO) DMA queue shallow so no transfer is ever urgent
        load_kq(0, 2)
        load_kq(1, 2)

        pend = None          # deferred rescale of the previous chain(b1, .)
        for n in range(1, QG):
            if n + 2 < QG:
                load_kq(0, n + 2)
                load_kq(1, n + 2)
            op0, fin0 = outproj_fillers(0, n - 1)
            f0 = with_rescale(merge_fillers(v_fillers(0, n), op0), pend)
            rb0 = attention_qg(0, n, f0)
            fin0()
            if n + 1 < QG:
                load_v(0, n + 1)
                load_v(1, n + 1)
            op1, fin1 = outproj_fillers(1, n - 1)
            if n + 1 < QG:
                f1 = with_rescale(
                    merge_fillers(v_fillers(1, n), op1,
                                  kq_fillers(0, n + 1) + kq_fillers(1, n + 1)),
                    rb0)
                op_fin0 = None
            else:
                # last iteration: rescale(b0,3) must land before the
                # outproj(b0,3) fillers, which are packed into the back
                # half of the chain (their staging/otall deps clear by then)
                op_last, op_fin0 = outproj_fillers(0, QG - 1)
                f1 = with_rescale(merge_fillers(v_fillers(1, n), op1), rb0,
                                  0.2) + op_last
            pend = attention_qg(1, n, f1)
            fin1()
            if op_fin0 is not None:
                op_fin0()
        pend()               # tail rescale(b1,3) runs immediately
        tail_fs, _ = outproj_fillers(1, QG - 1, tail=True)
        for f in tail_fs:
            f()

    nc.compile()
    return nc


def kernel(q, k, v, Wq, Wk, Wv, Wo):
    global LAST_RESULTS
    import ml_dtypes
    from concourse.bass_utils import run_bass_kernel_spmd

    bf16 = ml_dtypes.bfloat16

    def chunk_T(x):
        # [N, C] -> xT [C, N] -> [128, KCH, N] with x[p, kk, t] = xT[kk*128+p, t]
        xT = np.asarray(x, np.float32).reshape(N, C).T
        return np.ascontiguousarray(
            xT.reshape(KCH, 128, N).transpose(1, 0, 2)).astype(bf16)

    qc, kc, vc = chunk_T(q), chunk_T(k), chunk_T(v)
    Wq = np.asarray(Wq, np.float32)
    Wk = np.asarray(Wk, np.float32)
    Wv = np.asarray(Wv, np.float32)
    Wo = np.asarray(Wo, np.float32)

    def chunk_W(W, sl):
        # Wc = W[sl, :].T [C, LD] -> [128, KCH, LD]
        WT = W[sl, :].T
        return np.ascontiguousarray(
            WT.reshape(KCH, 128, LD).transpose(1, 0, 2)).astype(bf16)

    in_maps = []
    for c in range(NCORES):
        sl = slice(c * LD, (c + 1) * LD)
        in_maps.append({
            "qT": qc, "kT": kc, "vT": vc,
            "wqT": chunk_W(Wq, sl),
            "wkT": chunk_W(Wk, sl),
            "wvT": chunk_W(Wv, sl),
            "woT": np.ascontiguousarray(Wo[:, sl].T).astype(bf16),
        })

    nc = _build_program()
    res = run_bass_kernel_spmd(nc, in_maps, list(range(NCORES)))
    LAST_RESULTS = res
    acc = np.zeros((N, C), np.float32)
    for rmap in res.results:
        # device layout: out[p, blk, c] = row blk*128+p
        o = np.asarray(rmap["out"], np.float32)
        acc += o.transpose(1, 0, 2).reshape(N, C)
    return acc.reshape(B, T, C)
